# revision 5
# baseline (speedup 1.0000x reference)
"""MixHop layer (3 hops) on 8 Trainium2 NeuronCores.

out = concat_j [ adj_t^j @ (x @ W_j.T + b_j) ]   for j = 0,1,2

Strategy (destination sharding, one SPMD program on 8 cores), tuned to
minimize host<->device transfer over the axon tunnel (the wall-clock
bottleneck) and device DMA/Pool time (the on-chip bottleneck):
  - Each core receives ONLY its own x shard (fp16, host-transposed), the
    replicated [3,128,128] weights, and compact per-core edge encodings.
  - Phase AB: one pass over the shard computes y0 = x@W0.T+b0 (fp16 out)
    and the projection table shard [z1|z2] = x@[W1|W2].T+b (fp16).
  - AllGather (device, NeuronLink) assembles the full [N,256] fp16 table.
  - Phase C (SpMM1): dma_gather 512B fp16 table rows per in-edge
    (block-major chunk stream), build the one-hot*weight segment matrix S
    on device (tensor_scalar is_equal+mult against an iota tile),
    segment-sum via PE matmuls accumulated in PSUM.  Cols 0:128 -> out1,
    cols 128:256 -> z2 shard; both written with batched dma_scatter_add
    into pre-zeroed fp16 buffers.
  - Phase D: AllGather z2 shards -> full z2 table [N,128] fp16.
  - Phase E (SpMM2): same edge structure gathers z2 -> out2 (fp16).
Gather/scatter index streams are uploaded in compact [16, n/16] form and
replicated to the 128-partition layout on device; meta (dest slot, edge
weight) is fp16.  Outputs are fp16 (upcast on host); measured end-to-end
error vs the fp32 reference is ~5e-4 fro.
"""

import sys

sys.path.insert(0, "/opt/trn_rl_repo")

import heapq
import os

import numpy as np

import concourse.bass as bass
import concourse.tile as tile
from concourse import bacc, mybir
from concourse import bass_utils

P = 128


class Cfg:
    def __init__(self, n_nodes, n_feat, n_cores, k0max, k1max, gmax=8):
        assert n_nodes % n_cores == 0
        self.N = n_nodes
        self.F = n_feat
        self.NC = n_cores
        self.NS = n_nodes // n_cores          # dests per core
        self.NBLK = -(-self.NS // P)          # blocks per core
        self.K0 = k0max                       # window-0 chunks per block
        self.K1 = k1max                       # window-1 chunks per block
        self.K = k0max + k1max
        self.GMAX = gmax                      # chunks per dma_gather
        self.SGRP = 8                         # blocks per dma_scatter_add
        self.NSG = -(-self.NBLK // self.SGRP)
        self.NG0 = -(-(self.NBLK * k0max) // self.GMAX)   # win0 gathers/pass
        self.NG1 = -(-(self.NBLK * k1max) // self.GMAX)
        self.WIN = 32768 if n_nodes > 32768 else max(P, n_nodes // 2)


def _balanced_blocks(local_dest, ns, nblk):
    """Assign dests 0..ns-1 to nblk blocks of <=P slots, balancing edge
    counts.  Returns (block_of[ns], pos_of[ns], ids[P, nblk])."""
    deg = np.bincount(local_dest, minlength=ns)
    order = np.argsort(-deg, kind="stable")
    heap = [(0, 0, b) for b in range(nblk)]
    heapq.heapify(heap)
    block_of = np.empty(ns, np.int32)
    pos_of = np.empty(ns, np.int32)
    for d in order:
        while True:
            load, cnt, b = heapq.heappop(heap)
            if cnt < P:
                break
        block_of[d] = b
        pos_of[d] = cnt
        heapq.heappush(heap, (load + int(deg[d]), cnt + 1, b))
    # slot p of block b -> local output row (trash rows ns+p for empty slots)
    ids = np.empty((P, nblk), np.int32)
    for p in range(P):
        ids[p, :] = ns + p
    ids[pos_of, block_of] = np.arange(ns, dtype=np.int32)
    return block_of, pos_of, ids


def _precompute_core(r_loc, c_glob, w, cfg):
    ns, nblk = cfg.NS, cfg.NBLK
    block_of, pos_of, ids = _balanced_blocks(r_loc, ns, nblk)
    b_e = block_of[r_loc]
    dl_e = pos_of[r_loc]
    win_e = (c_glob >= cfg.WIN).astype(np.int64)
    order = np.lexsort((np.arange(len(r_loc)), win_e, b_e))
    b_s, win_s, dl_s, c_s, w_s = (
        b_e[order], win_e[order], dl_e[order], c_glob[order], w[order])
    key = b_s * 2 + win_s
    cnt = np.bincount(key, minlength=nblk * 2).reshape(nblk, 2)
    k0need = max(1, int(np.ceil(cnt[:, 0].max() / P))) if len(r_loc) else 1
    k1need = max(1, int(np.ceil(cnt[:, 1].max() / P))) if len(r_loc) else 1
    return dict(b=b_s, win=win_s, dl=dl_s, c=c_s, w=w_s, cnt=cnt, ids=ids,
                k0=k0need, k1=k1need)


def _encode_core(pc, cfg):
    """Device input arrays for one core, given global K0/K1.  Gather and
    scatter index streams are compact [16, n/16] (the device replicates to
    the 8 GPSIMD core groups)."""
    nblk, K0, K1, K = cfg.NBLK, cfg.K0, cfg.K1, cfg.K
    cnt = pc["cnt"]
    idx0 = np.zeros((nblk, K0 * P), np.int16)     # padded edge ids (win0)
    idx1 = np.zeros((nblk, K1 * P), np.int16)
    meta = np.zeros((P, nblk, K, 2), np.float16)  # (local dest, weight)
    starts = np.zeros(nblk * 2, np.int64)
    starts[1:] = np.cumsum(cnt.reshape(-1))[:-1]
    key = pc["b"] * 2 + pc["win"]
    iw = np.arange(len(key)) - starts[key]        # index within (b, win)
    b, win, dl, c, w = pc["b"], pc["win"], pc["dl"], pc["c"], pc["w"]
    m0 = win == 0
    idx0[b[m0], iw[m0]] = c[m0].astype(np.int16)
    m1 = ~m0
    idx1[b[m1], iw[m1]] = (c[m1] - cfg.WIN).astype(np.int16)
    kk = np.where(m0, iw // P, K0 + iw // P)
    meta[iw % P, b, kk, 0] = dl
    meta[iw % P, b, kk, 1] = w

    # global chunk-stream gather encodings [16, n_gath*GM*8]; dma_gather
    # reads logical id i from [i%16, i//16] of its idx window.
    GM = cfg.GMAX

    def enc(idx, Kw, n_gath):
        stream = idx.reshape(nblk * Kw * P)
        out = np.zeros((16, n_gath * GM * 8), np.int16)
        for g in range(n_gath):
            cg = min(GM, nblk * Kw - GM * g)
            flat = stream[g * GM * P: g * GM * P + cg * P]
            out[:, g * GM * 8: g * GM * 8 + cg * 8] = flat.reshape(-1, 16).T
        return out

    # batched scatter ids: group g covers SGRP blocks; logical i = c*128+p
    ids = pc["ids"]
    sid = np.zeros((16, cfg.NSG * cfg.SGRP * 8), np.int16)
    for g in range(cfg.NSG):
        nb = min(cfg.SGRP, nblk - g * cfg.SGRP)
        flat = ids[:, g * cfg.SGRP: g * cfg.SGRP + nb].T.reshape(-1)
        sid[:, g * cfg.SGRP * 8: g * cfg.SGRP * 8 + nb * 8] = (
            flat.reshape(-1, 16).T.astype(np.int16))
    return dict(
        idx0=enc(idx0, K0, cfg.NG0), idx1=enc(idx1, K1, cfg.NG1),
        meta=np.ascontiguousarray(meta.reshape(P, nblk * K * 2)),
        sid=sid,
    )


def _build_program(cfg, phases="ABCDE"):
    N, F, NC = cfg.N, cfg.F, cfg.NC
    NS, NBLK, K0, K1, K = cfg.NS, cfg.NBLK, cfg.K0, cfg.K1, cfg.K
    NW0 = min(N, cfg.WIN)
    NSP = NS + P                             # out buf rows incl trash
    f32 = mybir.dt.float32
    f16 = mybir.dt.float16
    GM, NG0, NG1 = cfg.GMAX, cfg.NG0, cfg.NG1
    SG, NSG = cfg.SGRP, cfg.NSG

    nc = bacc.Bacc("TRN2", target_bir_lowering=False, debug=False,
                   enable_asserts=False, num_devices=NC, num_swdge_queues=4)

    # ---- inputs ----------------------------------------------------------
    xsT = nc.dram_tensor("xsT", [F, NBLK * P], f16, kind="ExternalInput").ap()
    WT = nc.dram_tensor("WT", [3 * F, F], f16, kind="ExternalInput").ap()
    B16 = nc.dram_tensor("B16", [3, F], f16, kind="ExternalInput").ap()
    iota_in = nc.dram_tensor("iota", [P, P], f32, kind="ExternalInput").ap()
    idx0_in = nc.dram_tensor("idx0", [16, NG0 * GM * 8], mybir.dt.int16,
                             kind="ExternalInput").ap()
    idx1_in = nc.dram_tensor("idx1", [16, NG1 * GM * 8], mybir.dt.int16,
                             kind="ExternalInput").ap()
    meta_in = nc.dram_tensor("meta", [P, NBLK * K * 2], f16,
                             kind="ExternalInput").ap()
    sid_in = nc.dram_tensor("sid", [16, NSG * SG * 8], mybir.dt.int16,
                            kind="ExternalInput").ap()

    # ---- outputs / scratch ----------------------------------------------
    y0_buf = nc.dram_tensor("y0", [NBLK * P, F], f16, kind="ExternalOutput").ap()
    out1_buf = nc.dram_tensor("out1", [NSP, F], f16, kind="ExternalOutput").ap()
    out2_buf = nc.dram_tensor("out2", [NSP, F], f16, kind="ExternalOutput").ap()
    zsh = nc.dram_tensor("zsh", [NS, 2 * F], f16, kind="Internal").ap()
    table = nc.dram_tensor("table", [N, 2 * F], f16, kind="Internal",
                           addr_space="Shared").ap()
    z2s = nc.dram_tensor("z2s", [NSP, F], f16, kind="Internal").ap()
    z2t = nc.dram_tensor("z2t", [N, F], f16, kind="Internal",
                         addr_space="Shared").ap()

    qctr = [0]

    def next_queue():
        q = qctr[0] % 3
        qctr[0] += 1
        return q

    with tile.TileContext(nc) as tc:
        with tc.tile_pool(name="const", bufs=1) as cpool:
            iota_t = cpool.tile([P, P], f32)
            nc.sync.dma_start(iota_t[:], iota_in[:])
            meta16_t = cpool.tile([P, NBLK * K * 2], f16)
            nc.sync.dma_start(meta16_t[:], meta_in[:])
            meta_t = cpool.tile([P, NBLK * K * 2], f32)
            nc.vector.tensor_copy(meta_t[:], meta16_t[:])
            xs_t = cpool.tile([F, NBLK * P], f16)
            nc.sync.dma_start(xs_t[:], xsT[:])
            # compact idx streams: replicate [16, W] -> 8 partition groups
            ix0_t = cpool.tile([P, NG0 * GM * 8], mybir.dt.int16)
            ix1_t = cpool.tile([P, NG1 * GM * 8], mybir.dt.int16)
            sid_t = cpool.tile([P, NSG * SG * 8], mybir.dt.int16)
            for g in range(8):
                nc.sync.dma_start(ix0_t[16 * g:16 * (g + 1), :], idx0_in[:])
                nc.sync.dma_start(ix1_t[16 * g:16 * (g + 1), :], idx1_in[:])
                nc.sync.dma_start(sid_t[16 * g:16 * (g + 1), :], sid_in[:])
            wt_t = []
            b16_t = []
            for j in range(3):
                wtj = cpool.tile([F, F], f16, tag=f"wt{j}", name=f"wt{j}")
                b16j = cpool.tile([1, F], f16, tag=f"b16{j}", name=f"b16{j}")
                nc.sync.dma_start(wtj[:], WT[j * F:(j + 1) * F, :])
                nc.sync.dma_start(b16j[:], B16[j:j + 1, :])
                wt_t.append(wtj)
                b16_t.append(b16j)
            ones_t = cpool.tile([1, P], f16)
            nc.vector.memset(ones_t[:], 1.0)

            # ---- zero z2s (scatter-add base) -----------------------------
            if "C" in phases:
                with tc.tile_pool(name="zz", bufs=1) as zpool:
                    zt = zpool.tile([P, 2048], f16)
                    nc.vector.memset(zt[:], 0.0)
                    nrow = 0
                    while nrow + 2048 <= NSP:
                        nc.sync.dma_start(
                            z2s[nrow:nrow + 2048, :].rearrange(
                                "(a b) f -> a (b f)", a=P), zt[:])
                        nrow += 2048
                    while nrow + P <= NSP:
                        nc.sync.dma_start(
                            z2s[nrow:nrow + P, :].rearrange(
                                "(a b) f -> a (b f)", a=P), zt[:, :F])
                        nrow += P
                    assert nrow >= NS, (nrow, NS)

            # ---- Phase AB: project own shard with W0|W1|W2 ---------------
            # y0 (fp16) to y0_buf; [z1|z2] (fp16) to zsh for the AllGather.
            if "A" in phases:
             with tc.tile_pool(name="projAB", bufs=3) as apool, \
                  tc.tile_pool(name="psumAB", bufs=3, space="PSUM") as apsum:
                for t in range(NBLK):
                    r0 = t * P
                    r1 = min(NS, r0 + P)
                    w_ = r1 - r0
                    if w_ <= 0:
                        break
                    ps = apsum.tile([P, 3 * F], f32, space="PSUM")
                    for j in range(3):
                        nc.tensor.matmul(
                            ps[:w_, j * F:(j + 1) * F],
                            lhsT=xs_t[:, r0:r0 + w_], rhs=wt_t[j][:],
                            start=True, stop=False)
                        nc.tensor.matmul(
                            ps[:w_, j * F:(j + 1) * F],
                            lhsT=ones_t[:, :w_], rhs=b16_t[j][:],
                            start=False, stop=True)
                    st = apool.tile([P, 3 * F], f16, tag="stab")
                    if t % 2 == 0:
                        nc.vector.tensor_copy(st[:w_, :], ps[:w_, :])
                    else:
                        nc.scalar.copy(st[:w_, :], ps[:w_, :])
                    nc.sync.dma_start(y0_buf[r0:r1, :], st[:w_, 0:F])
                    nc.sync.dma_start(zsh[r0:r1, :], st[:w_, F:3 * F])

            # ---- Phase B: AllGather table shards -------------------------
            if "B" in phases:
                nc.gpsimd.collective_compute(
                    "AllGather", mybir.AluOpType.bypass,
                    replica_groups=[list(range(NC))],
                    ins=[zsh[:]], outs=[table[:]],
                )

            # ---- SpMM machinery ------------------------------------------
            def spmm(src_w0, src_w1, fdim, dst_bufs, gdt, stg_dts):
                """Gathers stream GM-chunk slices of the global block-major
                chunk stream per window; segment matmuls accumulate per
                block in PSUM; batched scatter-add to pre-zeroed buffers."""
                with tc.tile_pool(name="ga", bufs=4) as gapool, \
                     tc.tile_pool(name="sS", bufs=4) as spool, \
                     tc.tile_pool(name="stg", bufs=2) as stgpool, \
                     tc.tile_pool(name="psC", bufs=4, space="PSUM") as cpsum:
                    wins = [[src_w0, ix0_t, NBLK * K0, [], 0],
                            [src_w1, ix1_t, NBLK * K1, [], 0]]

                    def ensure_gathers(w, upto_chunk):
                        src_w, ix_t, tot, tiles, _ = wins[w]
                        while wins[w][4] * GM < min(upto_chunk, tot):
                            g = wins[w][4]
                            cg = min(GM, tot - GM * g)
                            ga = gapool.tile([P, GM, fdim], gdt,
                                             tag=f"ga{w}", name=f"ga{w}_{g}")
                            nc.gpsimd.dma_gather(
                                ga[:, :cg, :], src_w,
                                ix_t[:, g * GM * 8: g * GM * 8 + cg * 8],
                                num_idxs=cg * P, num_idxs_reg=cg * P,
                                elem_size=fdim, queue_num=next_queue())
                            tiles.append(ga)
                            wins[w][4] += 1

                    stgs = None
                    for b in range(NBLK):
                        g_s, c_s = b // SG, b % SG
                        nb = min(SG, NBLK - g_s * SG)
                        if c_s == 0:
                            stgs = [stgpool.tile([P, SG, F], stg_dts[i],
                                                 tag=f"stg{i}",
                                                 name=f"stg{i}_{g_s}")
                                    for i in range(len(dst_bufs))]
                        ensure_gathers(0, (b + 1) * K0)
                        ensure_gathers(1, (b + 1) * K1)
                        ps = cpsum.tile([P, fdim], f32, space="PSUM")
                        for k in range(K):
                            S = spool.tile([P, P], gdt, tag="S")
                            mo = (b * K + k) * 2
                            nc.vector.tensor_scalar(
                                out=S[:], in0=iota_t[:],
                                scalar1=meta_t[:, mo:mo + 1],
                                scalar2=meta_t[:, mo + 1:mo + 2],
                                op0=mybir.AluOpType.is_equal,
                                op1=mybir.AluOpType.mult)
                            if k < K0:
                                gk = b * K0 + k
                                rhs = wins[0][3][gk // GM][:, gk % GM, :]
                            else:
                                gk = b * K1 + (k - K0)
                                rhs = wins[1][3][gk // GM][:, gk % GM, :]
                            nc.tensor.matmul(ps[:], lhsT=S[:], rhs=rhs,
                                             start=(k == 0),
                                             stop=(k == K - 1))
                        for i, (dst, coff) in enumerate(dst_bufs):
                            if i % 2 == 0:
                                nc.vector.tensor_copy(stgs[i][:, c_s, :],
                                                      ps[:, coff:coff + F])
                            else:
                                nc.scalar.copy(stgs[i][:, c_s, :],
                                               ps[:, coff:coff + F])
                        if c_s == nb - 1:
                            for i, (dst, coff) in enumerate(dst_bufs):
                                nc.gpsimd.dma_scatter_add(
                                    dst, stgs[i][:, :nb, :],
                                    sid_t[:, g_s * SG * 8:
                                          g_s * SG * 8 + nb * 8],
                                    num_idxs=nb * P, num_idxs_reg=nb * P,
                                    elem_size=F, queue_num=3)

            # ---- Phase C: SpMM1 over table -> out1, z2s ------------------
            if "C" in phases:
                spmm(table[:NW0, :], table[cfg.WIN:N, :], 2 * F,
                     [(out1_buf[:], 0), (z2s[:], F)], f16, [f16, f16])

            # ---- Phase D: AllGather z2 shards ----------------------------
            if "D" in phases:
                nc.gpsimd.collective_compute(
                    "AllGather", mybir.AluOpType.bypass,
                    replica_groups=[list(range(NC))],
                    ins=[z2s[0:NS, :]], outs=[z2t[:]],
                )

            # ---- Phase E: SpMM2 over z2 table -> out2 --------------------
            if "E" in phases:
                spmm(z2t[:NW0, :], z2t[cfg.WIN:N, :], F,
                     [(out2_buf[:], 0)], f16, [f16])

    nc.compile()
    return nc


_CACHE = {}


def _get_program(cfg, phases="ABCDE"):
    key = (cfg.N, cfg.F, cfg.NC, cfg.K0, cfg.K1, cfg.GMAX, phases)
    if key not in _CACHE:
        _CACHE[key] = _build_program(cfg, phases)
    return _CACHE[key]


def _prepare(x, edge_weight, W, b, row, col, n_cores=8):
    N, F = np.asarray(x).shape
    row = np.asarray(row).astype(np.int64)
    col = np.asarray(col).astype(np.int64)
    w = np.asarray(edge_weight).astype(np.float32)
    x = np.asarray(x).astype(np.float32)
    W = np.asarray(W).astype(np.float32)
    b = np.asarray(b).astype(np.float32)

    ns = N // n_cores
    core_of = row // ns
    cfg0 = Cfg(N, F, n_cores, 1, 1)
    pcs = []
    for m in range(n_cores):
        sel = np.where(core_of == m)[0]
        pcs.append(_precompute_core(row[sel] - m * ns, col[sel], w[sel], cfg0))
    k0 = max(pc["k0"] for pc in pcs)
    k1 = max(pc["k1"] for pc in pcs)
    cfg = Cfg(N, F, n_cores, k0, k1)

    xT16 = x.T.astype(np.float16)                          # [F, N]
    WT = np.ascontiguousarray(
        np.transpose(W, (0, 2, 1))).reshape(3 * F, F).astype(np.float16)
    B16 = np.ascontiguousarray(b.astype(np.float16))       # [3, F]
    iota = np.tile(np.arange(P, dtype=np.float32), (P, 1))

    in_maps = []
    for m in range(n_cores):
        enc = _encode_core(pcs[m], cfg)
        xs = np.zeros((F, cfg.NBLK * P), np.float16)
        xs[:, :ns] = xT16[:, m * ns:(m + 1) * ns]
        in_maps.append(dict(
            xsT=xs, WT=WT, B16=B16, iota=iota,
            idx0=enc["idx0"], idx1=enc["idx1"], meta=enc["meta"],
            sid=enc["sid"],
        ))
    return cfg, in_maps


def kernel(x, edge_weight, W, b, row, col):
    n_cores = 8
    N, F = np.asarray(x).shape
    ns = N // n_cores
    cfg, in_maps = _prepare(x, edge_weight, W, b, row, col, n_cores)
    nc = _get_program(cfg)
    res = bass_utils.run_bass_kernel_spmd(nc, in_maps,
                                          core_ids=list(range(n_cores)))
    outs = []
    for m in range(n_cores):
        r = res.results[m]
        outs.append(np.concatenate(
            [r["y0"][:ns], r["out1"][:ns], r["out2"][:ns]], axis=1))
    return np.concatenate(outs, axis=0).astype(np.float32)


# revision 7
# speedup vs baseline: 1.0046x; 1.0046x over previous
"""MixHop layer (3 hops) on 8 Trainium2 NeuronCores.

out = concat_j [ adj_t^j @ (x @ W_j.T + b_j) ]   for j = 0,1,2

Strategy (destination sharding, one SPMD program on 8 cores), tuned to
minimize host<->device transfer over the axon tunnel (the wall-clock
bottleneck) and device DMA/Pool time (the on-chip bottleneck):
  - Each core receives ONLY its own x shard (fp16, host-transposed), the
    replicated [3,128,128] weights, and compact per-core edge encodings.
  - Phase AB: one pass over the shard computes y0 = x@W0.T+b0 (int8 out,
    per-row scales) and the projection shard [z1|z2] = x@[W1|W2].T+b (fp16).
  - AllGather (device, NeuronLink) assembles the full [N,256] fp16 table.
  - Phase C (SpMM1): dma_gather 512B fp16 table rows per in-edge
    (block-major chunk stream), build the one-hot*weight segment matrix S
    on device (tensor_scalar is_equal+mult against an iota tile),
    segment-sum via PE matmuls accumulated in PSUM.  Per block the PSUM
    result is quantized to int8 (cols 0:128 -> out1 half of q12) and
    copied fp16 (cols 128:256 -> z2s); batched dma_scatter_add delivers
    both into pre-zeroed buffers.
  - Phase D: AllGather z2 shards -> full z2 table [N,128] fp16.
  - Phase E (SpMM2): same edge structure gathers z2 -> out2 (int8 half
    of q12, per-row scales).
Outputs are int8 with per-row absmax scales kept in block-slot layout
[128, 3*NBLK] (the host knows the block permutation and undoes it);
row-wise int8 adds ~6e-3 fro error vs the fp32 reference -- well inside
the 2e-2 gate.  Gather/scatter index streams upload in compact [16, n/16]
form and are replicated to the 128-partition layout on device.
"""

import sys

sys.path.insert(0, "/opt/trn_rl_repo")

import heapq
import os

import numpy as np

import concourse.bass as bass
import concourse.tile as tile
from concourse import bacc, mybir
from concourse import bass_utils

P = 128
QMAX = 126.5


class Cfg:
    def __init__(self, n_nodes, n_feat, n_cores, k0max, k1max, gmax=8):
        assert n_nodes % n_cores == 0
        self.N = n_nodes
        self.F = n_feat
        self.NC = n_cores
        self.NS = n_nodes // n_cores          # dests per core
        self.NBLK = -(-self.NS // P)          # blocks per core
        self.K0 = k0max                       # window-0 chunks per block
        self.K1 = k1max                       # window-1 chunks per block
        self.K = k0max + k1max
        self.GMAX = gmax                      # chunks per dma_gather
        self.SGRP = 8                         # blocks per dma_scatter_add
        self.NSG = -(-self.NBLK // self.SGRP)
        self.NG0 = -(-(self.NBLK * k0max) // self.GMAX)   # win0 gathers/pass
        self.NG1 = -(-(self.NBLK * k1max) // self.GMAX)
        self.WIN = 32768 if n_nodes > 32768 else max(P, n_nodes // 2)


def _balanced_blocks(local_dest, ns, nblk):
    """Assign dests 0..ns-1 to nblk blocks of <=P slots, balancing edge
    counts.  Returns (block_of[ns], pos_of[ns], ids[P, nblk])."""
    deg = np.bincount(local_dest, minlength=ns)
    order = np.argsort(-deg, kind="stable")
    heap = [(0, 0, b) for b in range(nblk)]
    heapq.heapify(heap)
    block_of = np.empty(ns, np.int32)
    pos_of = np.empty(ns, np.int32)
    for d in order:
        while True:
            load, cnt, b = heapq.heappop(heap)
            if cnt < P:
                break
        block_of[d] = b
        pos_of[d] = cnt
        heapq.heappush(heap, (load + int(deg[d]), cnt + 1, b))
    # slot p of block b -> local output row (trash rows ns+p for empty slots)
    ids = np.empty((P, nblk), np.int32)
    for p in range(P):
        ids[p, :] = ns + p
    ids[pos_of, block_of] = np.arange(ns, dtype=np.int32)
    return block_of, pos_of, ids


def _precompute_core(r_loc, c_glob, w, cfg):
    ns, nblk = cfg.NS, cfg.NBLK
    block_of, pos_of, ids = _balanced_blocks(r_loc, ns, nblk)
    b_e = block_of[r_loc]
    dl_e = pos_of[r_loc]
    win_e = (c_glob >= cfg.WIN).astype(np.int64)
    order = np.lexsort((np.arange(len(r_loc)), win_e, b_e))
    b_s, win_s, dl_s, c_s, w_s = (
        b_e[order], win_e[order], dl_e[order], c_glob[order], w[order])
    key = b_s * 2 + win_s
    cnt = np.bincount(key, minlength=nblk * 2).reshape(nblk, 2)
    k0need = max(1, int(np.ceil(cnt[:, 0].max() / P))) if len(r_loc) else 1
    k1need = max(1, int(np.ceil(cnt[:, 1].max() / P))) if len(r_loc) else 1
    return dict(b=b_s, win=win_s, dl=dl_s, c=c_s, w=w_s, cnt=cnt, ids=ids,
                bo=block_of, po=pos_of, k0=k0need, k1=k1need)


def _encode_core(pc, cfg):
    """Device input arrays for one core, given global K0/K1.  Gather and
    scatter index streams are compact [16, n/16] (the device replicates to
    the 8 GPSIMD core groups)."""
    nblk, K0, K1, K = cfg.NBLK, cfg.K0, cfg.K1, cfg.K
    cnt = pc["cnt"]
    idx0 = np.zeros((nblk, K0 * P), np.int16)     # padded edge ids (win0)
    idx1 = np.zeros((nblk, K1 * P), np.int16)
    meta = np.zeros((P, nblk, K, 2), np.float16)  # (local dest, weight)
    starts = np.zeros(nblk * 2, np.int64)
    starts[1:] = np.cumsum(cnt.reshape(-1))[:-1]
    key = pc["b"] * 2 + pc["win"]
    iw = np.arange(len(key)) - starts[key]        # index within (b, win)
    b, win, dl, c, w = pc["b"], pc["win"], pc["dl"], pc["c"], pc["w"]
    m0 = win == 0
    idx0[b[m0], iw[m0]] = c[m0].astype(np.int16)
    m1 = ~m0
    idx1[b[m1], iw[m1]] = (c[m1] - cfg.WIN).astype(np.int16)
    kk = np.where(m0, iw // P, K0 + iw // P)
    meta[iw % P, b, kk, 0] = dl
    meta[iw % P, b, kk, 1] = w

    # global chunk-stream gather encodings [16, n_gath*GM*8]; dma_gather
    # reads logical id i from [i%16, i//16] of its idx window.
    GM = cfg.GMAX

    def enc(idx, Kw, n_gath):
        stream = idx.reshape(nblk * Kw * P)
        out = np.zeros((16, n_gath * GM * 8), np.int16)
        for g in range(n_gath):
            cg = min(GM, nblk * Kw - GM * g)
            flat = stream[g * GM * P: g * GM * P + cg * P]
            out[:, g * GM * 8: g * GM * 8 + cg * 8] = flat.reshape(-1, 16).T
        return out

    # batched scatter ids: group g covers SGRP blocks; logical i = c*128+p
    ids = pc["ids"]
    sid = np.zeros((16, cfg.NSG * cfg.SGRP * 8), np.int16)
    for g in range(cfg.NSG):
        nb = min(cfg.SGRP, nblk - g * cfg.SGRP)
        flat = ids[:, g * cfg.SGRP: g * cfg.SGRP + nb].T.reshape(-1)
        sid[:, g * cfg.SGRP * 8: g * cfg.SGRP * 8 + nb * 8] = (
            flat.reshape(-1, 16).T.astype(np.int16))
    return dict(
        idx0=enc(idx0, K0, cfg.NG0), idx1=enc(idx1, K1, cfg.NG1),
        meta=np.ascontiguousarray(meta.reshape(P, nblk * K * 2)),
        sid=sid,
    )


def _build_program(cfg, phases="ABCDE"):
    N, F, NC = cfg.N, cfg.F, cfg.NC
    NS, NBLK, K0, K1, K = cfg.NS, cfg.NBLK, cfg.K0, cfg.K1, cfg.K
    NW0 = min(N, cfg.WIN)
    NSP = NS + P                             # scatter buf rows incl trash
    f32 = mybir.dt.float32
    f16 = mybir.dt.float16
    i8 = mybir.dt.int8
    GM, NG0, NG1 = cfg.GMAX, cfg.NG0, cfg.NG1
    SG, NSG = cfg.SGRP, cfg.NSG

    nc = bacc.Bacc("TRN2", target_bir_lowering=False, debug=False,
                   enable_asserts=False, num_devices=NC, num_swdge_queues=4)

    # ---- inputs ----------------------------------------------------------
    xsT = nc.dram_tensor("xsT", [F, NBLK * P], f16, kind="ExternalInput").ap()
    WT = nc.dram_tensor("WT", [3 * F, F], f16, kind="ExternalInput").ap()
    B16 = nc.dram_tensor("B16", [3, F], f16, kind="ExternalInput").ap()
    iota_in = nc.dram_tensor("iota", [P, P], f32, kind="ExternalInput").ap()
    idx0_in = nc.dram_tensor("idx0", [16, NG0 * GM * 8], mybir.dt.int16,
                             kind="ExternalInput").ap()
    idx1_in = nc.dram_tensor("idx1", [16, NG1 * GM * 8], mybir.dt.int16,
                             kind="ExternalInput").ap()
    meta_in = nc.dram_tensor("meta", [P, NBLK * K * 2], f16,
                             kind="ExternalInput").ap()
    sid_in = nc.dram_tensor("sid", [16, NSG * SG * 8], mybir.dt.int16,
                            kind="ExternalInput").ap()

    # ---- outputs / scratch ----------------------------------------------
    qy_buf = nc.dram_tensor("qy", [NBLK * P, F], i8, kind="ExternalOutput").ap()
    q12_buf = nc.dram_tensor("q12", [NSP, 2 * F], i8,
                             kind="ExternalOutput").ap()
    sca_buf = nc.dram_tensor("sca", [P, 3 * NBLK], f16,
                             kind="ExternalOutput").ap()
    zsh = nc.dram_tensor("zsh", [NS, 2 * F], f16, kind="Internal").ap()
    table = nc.dram_tensor("table", [N, 2 * F], f16, kind="Internal",
                           addr_space="Shared").ap()
    z2s = nc.dram_tensor("z2s", [NSP, F], f16, kind="Internal").ap()
    z2t = nc.dram_tensor("z2t", [N, F], f16, kind="Internal",
                         addr_space="Shared").ap()

    qctr = [0]

    def next_queue():
        q = qctr[0] % 3
        qctr[0] += 1
        return q

    with tile.TileContext(nc) as tc:
        with tc.tile_pool(name="const", bufs=1) as cpool, \
             tc.tile_pool(name="rs", bufs=4) as rpool:
            iota_t = cpool.tile([P, P], f32)
            nc.sync.dma_start(iota_t[:], iota_in[:])
            meta16_t = cpool.tile([P, NBLK * K * 2], f16)
            nc.sync.dma_start(meta16_t[:], meta_in[:])
            meta_t = cpool.tile([P, NBLK * K * 2], f32)
            nc.vector.tensor_copy(meta_t[:], meta16_t[:])
            xs_t = cpool.tile([F, NBLK * P], f16)
            nc.sync.dma_start(xs_t[:], xsT[:])
            sca_t = cpool.tile([P, 3 * NBLK], f16)
            nc.vector.memset(sca_t[:], 0.0)
            # compact idx streams: replicate [16, W] -> 8 partition groups
            ix0_t = cpool.tile([P, NG0 * GM * 8], mybir.dt.int16)
            ix1_t = cpool.tile([P, NG1 * GM * 8], mybir.dt.int16)
            sid_t = cpool.tile([P, NSG * SG * 8], mybir.dt.int16)
            for g in range(8):
                nc.sync.dma_start(ix0_t[16 * g:16 * (g + 1), :], idx0_in[:])
                nc.sync.dma_start(ix1_t[16 * g:16 * (g + 1), :], idx1_in[:])
                nc.sync.dma_start(sid_t[16 * g:16 * (g + 1), :], sid_in[:])
            wt_t = []
            b16_t = []
            for j in range(3):
                wtj = cpool.tile([F, F], f16, tag=f"wt{j}", name=f"wt{j}")
                b16j = cpool.tile([1, F], f16, tag=f"b16{j}", name=f"b16{j}")
                nc.sync.dma_start(wtj[:], WT[j * F:(j + 1) * F, :])
                nc.sync.dma_start(b16j[:], B16[j:j + 1, :])
                wt_t.append(wtj)
                b16_t.append(b16j)
            ones_t = cpool.tile([1, P], f16)
            nc.vector.memset(ones_t[:], 1.0)

            def quantize(ps_slice, w_, scol, qout):
                """abs-rowmax -> sca_t[:, scol], qout = int8(ps*QMAX/rmax)."""
                nc.vector.tensor_reduce(
                    out=sca_t[:w_, scol:scol + 1], in_=ps_slice,
                    axis=mybir.AxisListType.X, op=mybir.AluOpType.max,
                    apply_absolute_value=True)
                rs = rpool.tile([P, 1], f32, tag="rs")
                nc.vector.tensor_scalar(
                    out=rs[:w_], in0=sca_t[:w_, scol:scol + 1],
                    scalar1=1e-20, scalar2=None, op0=mybir.AluOpType.max)
                nc.vector.reciprocal(rs[:w_], rs[:w_])
                nc.vector.tensor_scalar(
                    out=qout, in0=ps_slice, scalar1=rs[:w_], scalar2=QMAX,
                    op0=mybir.AluOpType.mult, op1=mybir.AluOpType.mult)

            # ---- zero z2s (scatter-add base) -----------------------------
            if "C" in phases:
                with tc.tile_pool(name="zz", bufs=1) as zpool:
                    zt = zpool.tile([P, 2048], f16)
                    nc.vector.memset(zt[:], 0.0)
                    nrow = 0
                    while nrow + 2048 <= NSP:
                        nc.sync.dma_start(
                            z2s[nrow:nrow + 2048, :].rearrange(
                                "(a b) f -> a (b f)", a=P), zt[:])
                        nrow += 2048
                    while nrow + P <= NSP:
                        nc.sync.dma_start(
                            z2s[nrow:nrow + P, :].rearrange(
                                "(a b) f -> a (b f)", a=P), zt[:, :F])
                        nrow += P
                    assert nrow >= NS, (nrow, NS)

            # ---- Phase AB: project own shard with W0|W1|W2 ---------------
            # y0 (int8 + scales) to qy_buf; [z1|z2] (fp16) to zsh.
            if "A" in phases:
             with tc.tile_pool(name="projAB", bufs=3) as apool, \
                  tc.tile_pool(name="psumAB", bufs=3, space="PSUM") as apsum:
                for t in range(NBLK):
                    r0 = t * P
                    r1 = min(NS, r0 + P)
                    w_ = r1 - r0
                    if w_ <= 0:
                        break
                    ps = apsum.tile([P, 3 * F], f32, space="PSUM")
                    for j in range(3):
                        nc.tensor.matmul(
                            ps[:w_, j * F:(j + 1) * F],
                            lhsT=xs_t[:, r0:r0 + w_], rhs=wt_t[j][:],
                            start=True, stop=False)
                        nc.tensor.matmul(
                            ps[:w_, j * F:(j + 1) * F],
                            lhsT=ones_t[:, :w_], rhs=b16_t[j][:],
                            start=False, stop=True)
                    qt = apool.tile([P, F], i8, tag="qt")
                    quantize(ps[:w_, 0:F], w_, t, qt[:w_, :])
                    nc.sync.dma_start(qy_buf[r0:r1, :], qt[:w_, :])
                    st = apool.tile([P, 2 * F], f16, tag="stab")
                    if t % 2 == 0:
                        nc.vector.tensor_copy(st[:w_, :], ps[:w_, F:3 * F])
                    else:
                        nc.scalar.copy(st[:w_, :], ps[:w_, F:3 * F])
                    nc.sync.dma_start(zsh[r0:r1, :], st[:w_, :])

            # ---- Phase B: AllGather table shards -------------------------
            if "B" in phases:
                nc.gpsimd.collective_compute(
                    "AllGather", mybir.AluOpType.bypass,
                    replica_groups=[list(range(NC))],
                    ins=[zsh[:]], outs=[table[:]],
                )

            # ---- SpMM machinery ------------------------------------------
            def spmm(src_w0, src_w1, fdim, dsts, scol0):
                """Gathers stream GM-chunk slices of the global block-major
                chunk stream per window; segment matmuls accumulate per
                block in PSUM; per-block int8 quantization (+ fp16 copy for
                z2); batched dma_scatter_add into pre-zeroed buffers.

                dsts: list of ("quant", out_col_ap) / ("f16", out_ap);
                col offset into PSUM is i*F."""
                with tc.tile_pool(name="ga", bufs=4) as gapool, \
                     tc.tile_pool(name="sS", bufs=8) as spool, \
                     tc.tile_pool(name="stg", bufs=2) as stgpool, \
                     tc.tile_pool(name="psC", bufs=4, space="PSUM") as cpsum:
                    wins = [[src_w0, ix0_t, NBLK * K0, [], 0],
                            [src_w1, ix1_t, NBLK * K1, [], 0]]

                    def ensure_gathers(w, upto_chunk):
                        src_w, ix_t, tot, tiles, _ = wins[w]
                        while wins[w][4] * GM < min(upto_chunk, tot):
                            g = wins[w][4]
                            cg = min(GM, tot - GM * g)
                            ga = gapool.tile([P, GM, fdim], f16,
                                             tag=f"ga{w}", name=f"ga{w}_{g}")
                            nc.gpsimd.dma_gather(
                                ga[:, :cg, :], src_w,
                                ix_t[:, g * GM * 8: g * GM * 8 + cg * 8],
                                num_idxs=cg * P, num_idxs_reg=cg * P,
                                elem_size=fdim, queue_num=next_queue())
                            tiles.append(ga)
                            wins[w][4] += 1

                    stgs = None
                    for b in range(NBLK):
                        g_s, c_s = b // SG, b % SG
                        nb = min(SG, NBLK - g_s * SG)
                        if c_s == 0:
                            stgs = [stgpool.tile(
                                        [P, SG, F],
                                        i8 if kind == "quant" else f16,
                                        tag=f"stg{i}", name=f"stg{i}_{g_s}")
                                    for i, (kind, _) in enumerate(dsts)]
                        ensure_gathers(0, (b + 1) * K0)
                        ensure_gathers(1, (b + 1) * K1)
                        ps = cpsum.tile([P, fdim], f32, space="PSUM")
                        for k in range(K):
                            S = spool.tile([P, P], f16, tag="S")
                            mo = (b * K + k) * 2
                            nc.vector.tensor_scalar(
                                out=S[:], in0=iota_t[:],
                                scalar1=meta_t[:, mo:mo + 1],
                                scalar2=meta_t[:, mo + 1:mo + 2],
                                op0=mybir.AluOpType.is_equal,
                                op1=mybir.AluOpType.mult)
                            if k < K0:
                                gk = b * K0 + k
                                rhs = wins[0][3][gk // GM][:, gk % GM, :]
                            else:
                                gk = b * K1 + (k - K0)
                                rhs = wins[1][3][gk // GM][:, gk % GM, :]
                            nc.tensor.matmul(ps[:], lhsT=S[:], rhs=rhs,
                                             start=(k == 0),
                                             stop=(k == K - 1))
                        for i, (kind, dst) in enumerate(dsts):
                            if kind == "quant":
                                quantize(ps[:, i * F:(i + 1) * F], P,
                                         scol0 + b, stgs[i][:, c_s, :])
                            else:
                                nc.scalar.copy(stgs[i][:, c_s, :],
                                               ps[:, i * F:(i + 1) * F])
                        if c_s == nb - 1:
                            for i, (kind, dst) in enumerate(dsts):
                                nc.gpsimd.dma_scatter_add(
                                    dst, stgs[i][:, :nb, :],
                                    sid_t[:, g_s * SG * 8:
                                          g_s * SG * 8 + nb * 8],
                                    num_idxs=nb * P, num_idxs_reg=nb * P,
                                    elem_size=F,
                                    elem_step=(2 * F if kind == "quant"
                                               else None),
                                    queue_num=3)

            # ---- Phase C: SpMM1 over table -> q12[:, :F], z2s ------------
            if "C" in phases:
                spmm(table[:NW0, :], table[cfg.WIN:N, :], 2 * F,
                     [("quant", q12_buf[:, 0:F]), ("f16", z2s[:])], NBLK)

            # ---- Phase D: AllGather z2 shards ----------------------------
            if "D" in phases:
                nc.gpsimd.collective_compute(
                    "AllGather", mybir.AluOpType.bypass,
                    replica_groups=[list(range(NC))],
                    ins=[z2s[0:NS, :]], outs=[z2t[:]],
                )

            # ---- Phase E: SpMM2 over z2 table -> q12[:, F:2F] ------------
            if "E" in phases:
                spmm(z2t[:NW0, :], z2t[cfg.WIN:N, :], F,
                     [("quant", q12_buf[:, F:2 * F])], 2 * NBLK)

            # ---- scales out ----------------------------------------------
            nc.sync.dma_start(sca_buf[:], sca_t[:])

    nc.compile()
    return nc


_CACHE = {}


def _get_program(cfg, phases="ABCDE"):
    key = (cfg.N, cfg.F, cfg.NC, cfg.K0, cfg.K1, cfg.GMAX, phases)
    if key not in _CACHE:
        _CACHE[key] = _build_program(cfg, phases)
    return _CACHE[key]


def _prepare(x, edge_weight, W, b, row, col, n_cores=8):
    N, F = np.asarray(x).shape
    row = np.asarray(row).astype(np.int64)
    col = np.asarray(col).astype(np.int64)
    w = np.asarray(edge_weight).astype(np.float32)
    x = np.asarray(x).astype(np.float32)
    W = np.asarray(W).astype(np.float32)
    b = np.asarray(b).astype(np.float32)

    ns = N // n_cores
    core_of = row // ns
    cfg0 = Cfg(N, F, n_cores, 1, 1)
    pcs = []
    for m in range(n_cores):
        sel = np.where(core_of == m)[0]
        pcs.append(_precompute_core(row[sel] - m * ns, col[sel], w[sel], cfg0))
    k0 = max(pc["k0"] for pc in pcs)
    k1 = max(pc["k1"] for pc in pcs)
    cfg = Cfg(N, F, n_cores, k0, k1)

    xT16 = x.T.astype(np.float16)                          # [F, N]
    WT = np.ascontiguousarray(
        np.transpose(W, (0, 2, 1))).reshape(3 * F, F).astype(np.float16)
    B16 = np.ascontiguousarray(b.astype(np.float16))       # [3, F]
    iota = np.tile(np.arange(P, dtype=np.float32), (P, 1))

    in_maps = []
    for m in range(n_cores):
        enc = _encode_core(pcs[m], cfg)
        xs = np.zeros((F, cfg.NBLK * P), np.float16)
        xs[:, :ns] = xT16[:, m * ns:(m + 1) * ns]
        in_maps.append(dict(
            xsT=xs, WT=WT, B16=B16, iota=iota,
            idx0=enc["idx0"], idx1=enc["idx1"], meta=enc["meta"],
            sid=enc["sid"],
        ))
    return cfg, in_maps, pcs


def kernel(x, edge_weight, W, b, row, col):
    n_cores = 8
    N, F = np.asarray(x).shape
    ns = N // n_cores
    cfg, in_maps, pcs = _prepare(x, edge_weight, W, b, row, col, n_cores)
    nc = _get_program(cfg)
    res = bass_utils.run_bass_kernel_spmd(nc, in_maps,
                                          core_ids=list(range(n_cores)))
    NBLK = cfg.NBLK
    outs = []
    for m in range(n_cores):
        r = res.results[m]
        sca = r["sca"].astype(np.float32) / QMAX     # [P, 3*NBLK]
        # y0 rows are tile-sequential: row r -> (slot r%P, tile r//P)
        s0 = sca[:, 0:NBLK].T.reshape(-1)[:ns]
        y0 = r["qy"][:ns].astype(np.float32) * s0[:, None]
        bo, po = pcs[m]["bo"], pcs[m]["po"]
        s1 = sca[po, NBLK + bo]
        out1 = r["q12"][:ns, 0:F].astype(np.float32) * s1[:, None]
        s2 = sca[po, 2 * NBLK + bo]
        out2 = r["q12"][:ns, F:2 * F].astype(np.float32) * s2[:, None]
        outs.append(np.concatenate([y0, out1, out2], axis=1))
    return np.concatenate(outs, axis=0).astype(np.float32)


# revision 13
# speedup vs baseline: 1.0263x; 1.0216x over previous
"""MixHop layer (3 hops) on 8 Trainium2 NeuronCores.

out = concat_j [ adj_t^j @ (x @ W_j.T + b_j) ]   for j = 0,1,2

Strategy (destination sharding, one SPMD program on 8 cores), tuned to
minimize host<->device transfer over the axon tunnel (the wall-clock
bottleneck) and device DMA/Pool time (the on-chip bottleneck):
  - Each core receives ONLY its own x shard (fp16, host-transposed), the
    replicated [3,128,128] weights, and compact per-core edge encodings.
  - Phase AB: one pass over the shard computes y0 = x@W0.T+b0 (int8 out,
    per-row scales) and the projection shard [z1|z2] = x@[W1|W2].T+b (fp16).
  - AllGather (device, NeuronLink) assembles the full [N,256] fp16 table.
  - Phase C (SpMM1): dma_gather 512B fp16 table rows per in-edge
    (block-major chunk stream), build the one-hot*weight segment matrix S
    on device (tensor_scalar is_equal+mult against an fp16 iota tile, DVE
    2x mode), segment-sum via PE matmuls accumulated in PSUM.  Per block
    the PSUM result is quantized to int8 (-> q12 cols 0:128) and copied
    fp16 (-> z2 block buffer); both are plain block-order DMA writes --
    no scatter: outputs stay in block-slot order and the HOST undoes the
    permutation (it computed it), while pass 2's gather indices are
    encoded directly in the block-permuted z2 coordinate system.
  - Phase D: AllGather z2 block buffers -> permuted z2 table [NC*NBLK*128].
  - Phase E (SpMM2): gathers permuted z2 rows -> out2 (int8, q12 cols
    128:256, per-row scales).
Outputs are int8 with per-row absmax scales in block-slot layout
[128, 3*NBLK]; row-wise int8 adds ~6e-3 fro error vs the fp32 reference,
well inside the 2e-2 gate.  Gather index streams upload in compact
[16, n/16] form and are replicated to the 128-partition layout on device.
"""

import sys

sys.path.insert(0, "/opt/trn_rl_repo")

import heapq
import os

import numpy as np

import concourse.bass as bass
import concourse.tile as tile
from concourse import bacc, mybir
from concourse import bass_utils

P = 128
QMAX = 126.5


class Cfg:
    def __init__(self, n_nodes, n_feat, n_cores, kk, gmax=8):
        assert n_nodes % n_cores == 0
        self.N = n_nodes
        self.F = n_feat
        self.NC = n_cores
        self.NS = n_nodes // n_cores          # dests per core
        self.NBLK = -(-self.NS // P)          # blocks per core
        self.NPB = n_cores * self.NBLK * P    # permuted z2 table rows
        # kk = (K0a, K1a, K0b, K1b): win chunks per block, passes 1 and 2
        self.KK = kk
        self.GMAX = gmax                      # chunks per dma_gather
        self.SGRP = 8                         # blocks per staging group
        self.NSG = -(-self.NBLK // self.SGRP)
        self.NG = tuple(-(-(self.NBLK * k) // gmax) for k in kk)
        self.WIN = 32768 if n_nodes > 32768 else max(P, n_nodes // 2)


def _balanced_blocks(local_dest, ns, nblk):
    """Assign dests 0..ns-1 to nblk blocks of <=P slots, balancing edge
    counts.  Returns (block_of[ns], pos_of[ns])."""
    deg = np.bincount(local_dest, minlength=ns)
    order = np.argsort(-deg, kind="stable")
    heap = [(0, 0, b) for b in range(nblk)]
    heapq.heapify(heap)
    block_of = np.empty(ns, np.int32)
    pos_of = np.empty(ns, np.int32)
    for d in order:
        while True:
            load, cnt, b = heapq.heappop(heap)
            if cnt < P:
                break
        block_of[d] = b
        pos_of[d] = cnt
        heapq.heappush(heap, (load + int(deg[d]), cnt + 1, b))
    return block_of, pos_of


def _pass_layout(b_e, dl_e, cp, w, win_thr, nblk):
    """Sort edges by (block, window); count per-(block,window); return the
    sorted streams and needed chunk capacities."""
    win_e = (cp >= win_thr).astype(np.int64)
    order = np.lexsort((np.arange(len(b_e)), win_e, b_e))
    b_s, win_s, dl_s, c_s, w_s = (
        b_e[order], win_e[order], dl_e[order], cp[order], w[order])
    cnt = np.bincount(b_s * 2 + win_s, minlength=nblk * 2).reshape(nblk, 2)
    k0 = max(1, int(np.ceil(cnt[:, 0].max() / P))) if len(b_e) else 1
    k1 = max(1, int(np.ceil(cnt[:, 1].max() / P))) if len(b_e) else 1
    return dict(b=b_s, win=win_s, dl=dl_s, c=c_s, w=w_s, cnt=cnt,
                k0=k0, k1=k1)


def _encode_pass(pl, K0, K1, NG0, NG1, cfg):
    """idx0/idx1 [16, NG*GM*8] compact gather id streams + meta fp16."""
    nblk, K, GM = cfg.NBLK, K0 + K1, cfg.GMAX
    idx0 = np.zeros((nblk, K0 * P), np.int16)
    idx1 = np.zeros((nblk, K1 * P), np.int16)
    meta = np.zeros((P, nblk, K, 2), np.float16)
    cnt = pl["cnt"]
    starts = np.zeros(nblk * 2, np.int64)
    starts[1:] = np.cumsum(cnt.reshape(-1))[:-1]
    key = pl["b"] * 2 + pl["win"]
    iw = np.arange(len(key)) - starts[key]
    b, win, dl, c, w = pl["b"], pl["win"], pl["dl"], pl["c"], pl["w"]
    m0 = win == 0
    idx0[b[m0], iw[m0]] = c[m0].astype(np.int16)
    m1 = ~m0
    idx1[b[m1], iw[m1]] = (c[m1] - cfg.WIN).astype(np.int16)
    kk = np.where(m0, iw // P, K0 + iw // P)
    meta[iw % P, b, kk, 0] = dl
    meta[iw % P, b, kk, 1] = w

    def enc(idx, Kw, n_gath):
        stream = idx.reshape(nblk * Kw * P)
        out = np.zeros((16, n_gath * GM * 8), np.int16)
        for g in range(n_gath):
            cg = min(GM, nblk * Kw - GM * g)
            flat = stream[g * GM * P: g * GM * P + cg * P]
            out[:, g * GM * 8: g * GM * 8 + cg * 8] = flat.reshape(-1, 16).T
        return out

    return (enc(idx0, K0, NG0), enc(idx1, K1, NG1),
            np.ascontiguousarray(meta.reshape(P, nblk * K * 2)))


def _build_program(cfg, phases="ABCDE"):
    N, F, NC = cfg.N, cfg.F, cfg.NC
    NS, NBLK = cfg.NS, cfg.NBLK
    NPB = cfg.NPB
    K0a, K1a, K0b, K1b = cfg.KK
    Ka, Kb = K0a + K1a, K0b + K1b
    NGa0, NGa1, NGb0, NGb1 = cfg.NG
    NW0 = min(N, cfg.WIN)
    f32 = mybir.dt.float32
    f16 = mybir.dt.float16
    i8 = mybir.dt.int8
    GM = cfg.GMAX
    SG = cfg.SGRP

    nc = bacc.Bacc("TRN2", target_bir_lowering=False, debug=False,
                   enable_asserts=False, num_devices=NC, num_swdge_queues=4)

    # ---- inputs ----------------------------------------------------------
    xsT = nc.dram_tensor("xsT", [F, NBLK * P], f16, kind="ExternalInput").ap()
    WT = nc.dram_tensor("WT", [3 * F, F], f16, kind="ExternalInput").ap()
    B16 = nc.dram_tensor("B16", [3, F], f16, kind="ExternalInput").ap()
    ins = {}
    for nm, ng0, ng1, kk in (("a", NGa0, NGa1, Ka), ("b", NGb0, NGb1, Kb)):
        ins[f"idx0{nm}"] = nc.dram_tensor(
            f"idx0{nm}", [16, ng0 * GM * 8], mybir.dt.int16,
            kind="ExternalInput").ap()
        ins[f"idx1{nm}"] = nc.dram_tensor(
            f"idx1{nm}", [16, ng1 * GM * 8], mybir.dt.int16,
            kind="ExternalInput").ap()
        ins[f"meta{nm}"] = nc.dram_tensor(
            f"meta{nm}", [P, NBLK * kk * 2], f16, kind="ExternalInput").ap()

    # ---- outputs / scratch ----------------------------------------------
    qy_buf = nc.dram_tensor("qy", [NBLK * P, F], i8, kind="ExternalOutput").ap()
    q12_buf = nc.dram_tensor("q12", [NBLK * P, 2 * F], i8,
                             kind="ExternalOutput").ap()
    sca_buf = nc.dram_tensor("sca", [P, 3 * NBLK], f16,
                             kind="ExternalOutput").ap()
    zsh = nc.dram_tensor("zsh", [NS, 2 * F], f16, kind="Internal").ap()
    table = nc.dram_tensor("table", [N, 2 * F], f16, kind="Internal",
                           addr_space="Shared").ap()
    z2b = nc.dram_tensor("z2b", [NBLK * P, F], f16, kind="Internal").ap()
    z2t = nc.dram_tensor("z2t", [NPB, F], f16, kind="Internal",
                         addr_space="Shared").ap()

    qctr = [0]

    def next_queue():
        q = qctr[0] % 4
        qctr[0] += 1
        return q

    with tile.TileContext(nc) as tc:
        with tc.tile_pool(name="const", bufs=1) as cpool, \
             tc.tile_pool(name="rs", bufs=4) as rpool:
            iota_i = cpool.tile([P, P], mybir.dt.int16)
            nc.gpsimd.iota(iota_i[:], pattern=[[1, P]], base=0,
                           channel_multiplier=0)
            iota_t = cpool.tile([P, P], f16)
            nc.vector.tensor_copy(iota_t[:], iota_i[:])
            xs_t = cpool.tile([F, NBLK * P], f16)
            nc.sync.dma_start(xs_t[:], xsT[:])
            sca_t = cpool.tile([P, 3 * NBLK], f16)
            nc.vector.memset(sca_t[:], 0.0)
            tiles = {}
            for nm, ng0, ng1, kk in (("a", NGa0, NGa1, Ka),
                                     ("b", NGb0, NGb1, Kb)):
                m16 = cpool.tile([P, NBLK * kk * 2], f16, tag=f"m16{nm}",
                                 name=f"m16{nm}")
                nc.sync.dma_start(m16[:], ins[f"meta{nm}"][:])
                mt = cpool.tile([P, NBLK * kk * 2], f32, tag=f"mt{nm}",
                                name=f"mt{nm}")
                nc.vector.tensor_copy(mt[:], m16[:])
                tiles[f"meta{nm}"] = mt
                for w_, ng in (("0", ng0), ("1", ng1)):
                    ix = cpool.tile([P, ng * GM * 8], mybir.dt.int16,
                                    tag=f"ix{w_}{nm}", name=f"ix{w_}{nm}")
                    for g in range(8):
                        nc.sync.dma_start(ix[16 * g:16 * (g + 1), :],
                                          ins[f"idx{w_}{nm}"][:])
                    tiles[f"ix{w_}{nm}"] = ix
            wt_t = []
            b16_t = []
            for j in range(3):
                wtj = cpool.tile([F, F], f16, tag=f"wt{j}", name=f"wt{j}")
                b16j = cpool.tile([1, F], f16, tag=f"b16{j}", name=f"b16{j}")
                nc.sync.dma_start(wtj[:], WT[j * F:(j + 1) * F, :])
                nc.sync.dma_start(b16j[:], B16[j:j + 1, :])
                wt_t.append(wtj)
                b16_t.append(b16j)
            ones_t = cpool.tile([1, P], f16)
            nc.vector.memset(ones_t[:], 1.0)

            def quantize(ps_slice, w_, scol, qout):
                """abs-rowmax -> sca_t[:, scol], qout = int8(ps*QMAX/rmax)."""
                nc.vector.tensor_reduce(
                    out=sca_t[:w_, scol:scol + 1], in_=ps_slice,
                    axis=mybir.AxisListType.X, op=mybir.AluOpType.max,
                    apply_absolute_value=True)
                rs = rpool.tile([P, 1], f32, tag="rs")
                nc.vector.tensor_scalar(
                    out=rs[:w_], in0=sca_t[:w_, scol:scol + 1],
                    scalar1=1e-20, scalar2=None, op0=mybir.AluOpType.max)
                nc.vector.reciprocal(rs[:w_], rs[:w_])
                nc.vector.tensor_scalar(
                    out=qout, in0=ps_slice, scalar1=rs[:w_], scalar2=QMAX,
                    op0=mybir.AluOpType.mult, op1=mybir.AluOpType.mult)

            # ---- Phase AB: project own shard with W0|W1|W2 ---------------
            # y0 (int8 + scales) to qy_buf; [z1|z2] (fp16) to zsh.
            if "A" in phases:
             with tc.tile_pool(name="projAB", bufs=3) as apool, \
                  tc.tile_pool(name="psumAB", bufs=3, space="PSUM") as apsum:
                for t in range(NBLK):
                    r0 = t * P
                    r1 = min(NS, r0 + P)
                    w_ = r1 - r0
                    if w_ <= 0:
                        break
                    ps = apsum.tile([P, 3 * F], f32, space="PSUM")
                    for j in range(3):
                        nc.tensor.matmul(
                            ps[:w_, j * F:(j + 1) * F],
                            lhsT=xs_t[:, r0:r0 + w_], rhs=wt_t[j][:],
                            start=True, stop=False)
                        nc.tensor.matmul(
                            ps[:w_, j * F:(j + 1) * F],
                            lhsT=ones_t[:, :w_], rhs=b16_t[j][:],
                            start=False, stop=True)
                    qt = apool.tile([P, F], i8, tag="qt")
                    quantize(ps[:w_, 0:F], w_, t, qt[:w_, :])
                    nc.sync.dma_start(qy_buf[r0:r1, :], qt[:w_, :])
                    st = apool.tile([P, 2 * F], f16, tag="stab")
                    if t % 2 == 0:
                        nc.vector.tensor_copy(st[:w_, :], ps[:w_, F:3 * F])
                    else:
                        nc.scalar.copy(st[:w_, :], ps[:w_, F:3 * F])
                    nc.sync.dma_start(zsh[r0:r1, :], st[:w_, :])

            # ---- Phase B: AllGather table shards -------------------------
            if "B" in phases:
                nc.gpsimd.collective_compute(
                    "AllGather", mybir.AluOpType.bypass,
                    replica_groups=[list(range(NC))],
                    ins=[zsh[:]], outs=[table[:]],
                )

            # ---- SpMM machinery ------------------------------------------
            def spmm(src_w0, src_w1, fdim, dsts, scol0, ix0_t, ix1_t,
                     meta_t, K0, K1):
                """Gathers stream GM-chunk slices of the block-major chunk
                stream per window; segment matmuls accumulate per block in
                PSUM; per-block int8 quantization (+ fp16 copy for z2);
                block-order [P, SG*F] DMA writes (host/pass-2 indices undo
                the block permutation -- no scatter)."""
                K = K0 + K1
                with tc.tile_pool(name="ga", bufs=6) as gapool, \
                     tc.tile_pool(name="sS", bufs=32) as spool, \
                     tc.tile_pool(name="stg", bufs=3) as stgpool, \
                     tc.tile_pool(name="psC", bufs=4, space="PSUM") as cpsum:
                    wins = [[src_w0, ix0_t, NBLK * K0, [], 0],
                            [src_w1, ix1_t, NBLK * K1, [], 0]]

                    def ensure_gathers(w, upto_chunk):
                        src_w, ix_t, tot, gtiles, _ = wins[w]
                        while wins[w][4] * GM < min(upto_chunk, tot):
                            g = wins[w][4]
                            cg = min(GM, tot - GM * g)
                            ga = gapool.tile([P, GM, fdim], f16,
                                             tag=f"ga{w}", name=f"ga{w}_{g}")
                            nc.gpsimd.dma_gather(
                                ga[:, :cg, :], src_w,
                                ix_t[:, g * GM * 8: g * GM * 8 + cg * 8],
                                num_idxs=cg * P, num_idxs_reg=cg * P,
                                elem_size=fdim, queue_num=next_queue())
                            gtiles.append(ga)
                            wins[w][4] += 1

                    stgs = None
                    for b in range(NBLK):
                        g_s, c_s = b // SG, b % SG
                        nb = min(SG, NBLK - g_s * SG)
                        if c_s == 0:
                            stgs = [stgpool.tile(
                                        [P, SG, F],
                                        i8 if kind == "quant" else f16,
                                        tag=f"stg{i}", name=f"stg{i}_{g_s}")
                                    for i, (kind, _) in enumerate(dsts)]
                        ensure_gathers(0, (b + 1) * K0)
                        ensure_gathers(1, (b + 1) * K1)
                        ps = cpsum.tile([P, fdim], f32, space="PSUM")
                        for k in range(K):
                            S = spool.tile([P, P], f16, tag="S")
                            mo = (b * K + k) * 2
                            nc.vector.tensor_scalar(
                                out=S[:], in0=iota_t[:],
                                scalar1=meta_t[:, mo:mo + 1],
                                scalar2=meta_t[:, mo + 1:mo + 2],
                                op0=mybir.AluOpType.is_equal,
                                op1=mybir.AluOpType.mult)
                            if k < K0:
                                gk = b * K0 + k
                                rhs = wins[0][3][gk // GM][:, gk % GM, :]
                            else:
                                gk = b * K1 + (k - K0)
                                rhs = wins[1][3][gk // GM][:, gk % GM, :]
                            nc.tensor.matmul(ps[:], lhsT=S[:], rhs=rhs,
                                             start=(k == 0),
                                             stop=(k == K - 1))
                        for i, (kind, dst) in enumerate(dsts):
                            if kind == "quant":
                                quantize(ps[:, i * F:(i + 1) * F], P,
                                         scol0 + b, stgs[i][:, c_s, :])
                            else:
                                nc.scalar.copy(stgs[i][:, c_s, :],
                                               ps[:, i * F:(i + 1) * F])
                        if c_s == nb - 1:
                            r0 = g_s * SG * P
                            r1 = r0 + nb * P
                            for i, (kind, dst) in enumerate(dsts):
                                nc.sync.dma_start(
                                    dst[r0:r1, :].rearrange(
                                        "(b p) f -> p b f", p=P),
                                    stgs[i][:, :nb, :])

            # ---- Phase C: SpMM1 over table -> q12[:, :F], z2b ------------
            if "C" in phases:
                spmm(table[:NW0, :], table[cfg.WIN:N, :], 2 * F,
                     [("quant", q12_buf[:, 0:F]), ("f16", z2b[:])], NBLK,
                     tiles["ix0a"], tiles["ix1a"], tiles["metaa"], K0a, K1a)

            # ---- Phase D: AllGather z2 block buffers ---------------------
            if "D" in phases:
                nc.gpsimd.collective_compute(
                    "AllGather", mybir.AluOpType.bypass,
                    replica_groups=[list(range(NC))],
                    ins=[z2b[:]], outs=[z2t[:]],
                )

            # ---- Phase E: SpMM2 over permuted z2 -> q12[:, F:2F] ---------
            if "E" in phases:
                spmm(z2t[:NW0, :], z2t[cfg.WIN:NPB, :], F,
                     [("quant", q12_buf[:, F:2 * F])], 2 * NBLK,
                     tiles["ix0b"], tiles["ix1b"], tiles["metab"], K0b, K1b)

            # ---- scales out ----------------------------------------------
            nc.sync.dma_start(sca_buf[:], sca_t[:])

    nc.compile()
    return nc


_CACHE = {}


def _get_program(cfg, phases="ABCDE"):
    key = (cfg.N, cfg.F, cfg.NC, cfg.KK, cfg.GMAX, phases)
    if key not in _CACHE:
        _CACHE[key] = _build_program(cfg, phases)
    return _CACHE[key]


def _prepare(x, edge_weight, W, b, row, col, n_cores=8):
    N, F = np.asarray(x).shape
    row = np.asarray(row).astype(np.int64)
    col = np.asarray(col).astype(np.int64)
    w = np.asarray(edge_weight).astype(np.float32)
    x = np.asarray(x).astype(np.float32)
    W = np.asarray(W).astype(np.float32)
    b = np.asarray(b).astype(np.float32)

    ns = N // n_cores
    nblk = -(-ns // P)
    core_of = row // ns

    # block assignment per core + permuted z2 id map
    edges = []
    bos, pos = [], []
    pmap = np.empty(N, np.int32)
    for m in range(n_cores):
        sel = np.where(core_of == m)[0]
        r_loc = (row[sel] - m * ns).astype(np.int64)
        bo, po = _balanced_blocks(r_loc, ns, nblk)
        bos.append(bo)
        pos.append(po)
        pmap[m * ns:(m + 1) * ns] = m * nblk * P + bo * P + po
        edges.append((bo[r_loc], po[r_loc], col[sel], w[sel]))

    win_thr = 32768 if N > 32768 else max(P, N // 2)
    pls_a, pls_b = [], []
    for m in range(n_cores):
        b_e, dl_e, c_e, w_e = edges[m]
        pls_a.append(_pass_layout(b_e, dl_e, c_e, w_e, win_thr, nblk))
        pls_b.append(_pass_layout(b_e, dl_e, pmap[c_e], w_e, win_thr, nblk))
    kk = (max(pl["k0"] for pl in pls_a), max(pl["k1"] for pl in pls_a),
          max(pl["k0"] for pl in pls_b), max(pl["k1"] for pl in pls_b))
    cfg = Cfg(N, F, n_cores, kk)

    xT16 = x.T.astype(np.float16)                          # [F, N]
    WT = np.ascontiguousarray(
        np.transpose(W, (0, 2, 1))).reshape(3 * F, F).astype(np.float16)
    B16 = np.ascontiguousarray(b.astype(np.float16))       # [3, F]

    NGa0, NGa1, NGb0, NGb1 = cfg.NG
    in_maps = []
    for m in range(n_cores):
        i0a, i1a, ma = _encode_pass(pls_a[m], kk[0], kk[1], NGa0, NGa1, cfg)
        i0b, i1b, mb = _encode_pass(pls_b[m], kk[2], kk[3], NGb0, NGb1, cfg)
        xs = np.zeros((F, cfg.NBLK * P), np.float16)
        xs[:, :ns] = xT16[:, m * ns:(m + 1) * ns]
        in_maps.append(dict(
            xsT=xs, WT=WT, B16=B16,
            idx0a=i0a, idx1a=i1a, metaa=ma,
            idx0b=i0b, idx1b=i1b, metab=mb,
        ))
    return cfg, in_maps, (bos, pos)


def kernel(x, edge_weight, W, b, row, col):
    n_cores = 8
    N, F = np.asarray(x).shape
    ns = N // n_cores
    cfg, in_maps, (bos, pos) = _prepare(x, edge_weight, W, b, row, col,
                                        n_cores)
    nc = _get_program(cfg)
    res = bass_utils.run_bass_kernel_spmd(nc, in_maps,
                                          core_ids=list(range(n_cores)))
    NBLK = cfg.NBLK
    outs = []
    for m in range(n_cores):
        r = res.results[m]
        sca = r["sca"].astype(np.float32) / QMAX     # [P, 3*NBLK]
        # y0 rows are tile-sequential: row r -> (slot r%P, tile r//P)
        s0 = sca[:, 0:NBLK].T.reshape(-1)[:ns]
        y0 = r["qy"][:ns].astype(np.float32) * s0[:, None]
        bo, po = bos[m], pos[m]
        slot = bo * P + po                           # block-permuted row
        q12 = r["q12"]
        out1 = q12[slot, 0:F].astype(np.float32) * sca[po, NBLK + bo][:, None]
        out2 = (q12[slot, F:2 * F].astype(np.float32)
                * sca[po, 2 * NBLK + bo][:, None])
        outs.append(np.concatenate([y0, out1, out2], axis=1))
    return np.concatenate(outs, axis=0).astype(np.float32)
